# revision 1
# baseline (speedup 1.0000x reference)
"""Self-contained Trainium2 Bass kernel: mean symmetric point-to-closest-point
(Chamfer) distance between batches of 2048-point 2D clouds.

Problem: outputs/targets (32, 4096) fp32 -> point clouds (32, 2048, 2);
result = mean_b 0.5*(mean_i min_j d_ij + mean_j min_i d_ij), a fp32 scalar.

Sharding: data parallel over the batch dim - core c computes batches
4c..4c+3; each core returns partial sums of sqrt(min d^2) in res[128, 1];
the host sums and scales (an all-reduce-mean equivalent done host-side
since the output is a scalar).

Device algorithm per core (4 batches):
  * D2[i,j] = ||u_i||^2 + ||v_j||^2 - 2 u_i.v_j is computed on the
    TensorEngine as a K=10 matmul with fp16 hi/lo-split operands
    (fp32-grade accuracy at full 1 cycle/row PE rate), 512 cols per
    PSUM bank, double-buffered across the 8 banks.
  * W/M pack-vector orders are chosen so the [10, 2048] operand rows
    form contiguous blocks of the PE-transposed pack: assembly is 7
    block-DMAs per batch instead of 12 row-scatters (and batch 0's
    gate-DMAs spread over 3 queues).
  * ScalarEngine evacuates each PSUM tile to SBUF fp16 with a fused
    Relu clamp, enabling DVE 2x packed-fp16 mode.
  * Row mins (u->v): paired-tile first-level folds into s1p2, then
    per-pair folds into a per-batch buffer finished by an in-place
    2x fold tree + one 1x reduce.
    Col mins (v->u): running TT-min accumulator (first op consumes the
    first two tiles directly - no init copy), finalized with PE
    transposes + one 1x reduce straight from PSUM (DVE can read only
    ONE non-scalar input from PSUM, so pairwise PSUM folds are illegal).
  * Each batch's rowmin-tree + colmin-finalize is deferred into the
    NEXT batch's pipeline (emitted at its t==2 slot) so batch
    boundaries never stall DVE; sqrt runs per batch, with one fused
    Copy+accum activation producing [128, 1] partials DMA'd out.
  * The rep loop (timing builds) runs TWO ping-pong body copies per
    For_i iteration ("A"/"B" tile slots): the loop's all-engine reset
    barrier fires once per two reps and slot B's prep overlaps slot
    A's main loop, amortizing pipeline fill/drain. Steady state is
    DVE-bound at ~93% occupancy (sim: 165.2us/rep; single body was
    171.4us); ScalarE 127us/rep, PE 67us/rep.

Notes from HW bring-up: DVE ops with accum_out (tensor_tensor_reduce,
tensor_scalar+accum) crash this environment's runtime (verified: device
becomes unrecoverable), GPSIMD tensor_tensor fails walrus codegen
(verified: ISA check rejects TT on Pool), and DMA accum_op=min is
rejected by the compiler (verified) - hence all reductions stay on DVE.
InstMax (vector.max top-8) works but runs at 1x, slower than 2x fold
chains. ScalarE activation accum (sum) works.
"""
from contextlib import ExitStack

import numpy as np

import concourse.bacc as bacc
import concourse.tile as tile
from concourse import mybir
from concourse.bass_utils import run_bass_kernel_spmd

F16 = mybir.dt.float16
F32 = mybir.dt.float32
MIN = mybir.AluOpType.min

N_CORES = 8
NB = 4          # batches per core
NPT = 2048      # points per cloud
NT = 16         # 128-point i-tiles per batch


def _emit_consts(nc, ident_d, ones_d, sing):
    """Constant loads hoisted out of the rep loop."""
    ident = sing.tile([128, 128], F16, name="ident")
    nc.sync.dma_start(out=ident, in_=ident_d[:, :])
    ones_sb = sing.tile([2, NPT], F16, name="ones_sb")
    nc.sync.dma_start(out=ones_sb, in_=ones_d[:, :])
    # Pre-warm the Sqrt activation-table set (whose fillers include Relu and
    # Copy) so no ACT_TABLE_LOAD ever fires mid-pipeline.
    warm = sing.tile([1, 1], F32, name="act_warm")
    nc.scalar.activation(warm, ones_sb[0:1, 0:1],
                         mybir.ActivationFunctionType.Sqrt)
    return ident, ones_sb


def _emit_prep(nc, out_d, tgt_d, ident, ones_sb, pools, slot=""):
    work, pp = pools

    # ---- load raw coords as [128, 4, 16]: i = p*16+g, one DMA per stream ----
    raw = {}
    for nm, dram, lo in (("ux", out_d, 0), ("uy", out_d, NPT),
                         ("vx", tgt_d, 0), ("vy", tgt_d, NPT)):
        t = work.tile([128, NB, 16], F32, name=f"raw_{nm}{slot}",
                      tag=f"raw_{nm}{slot}", bufs=1)
        eng = nc.sync if nm in ("ux", "vx") else nc.gpsimd
        eng.dma_start(
            out=t,
            in_=dram[:, lo:lo + NPT].rearrange("b (p g) -> p b g", g=16),
        )
        raw[nm] = t

    # ---- fp16 hi/lo splits at [128, 64] granularity ----
    # Vector orders are chosen so W/M rows form contiguous blocks of the
    # transposed pack, collapsing the row-scatter DMAs into block DMAs.
    # pack_u vectors: 0 nu_hi, 1 nu_lo, 2 uxlo, 3 uylo, 4 uxhi, 5 uyhi
    # pack_v vectors: 0 -2vxhi, 1 -2vyhi, 2 nv_hi, 3 nv_lo, 4 -2vxlo, 5 -2vylo
    pack_u = work.tile([128, NB, 6, 16], F16, name=f"pack_u{slot}",
                       tag=f"pku{slot}", bufs=1)
    pack_v = work.tile([128, NB, 6, 16], F16, name=f"pack_v{slot}",
                       tag=f"pkv{slot}", bufs=1)

    for side, (cx, cy), pack in (("u", ("ux", "uy"), pack_u),
                                 ("v", ("vx", "vy"), pack_v)):
        x = raw[cx].rearrange("p b g -> p (b g)")
        y = raw[cy].rearrange("p b g -> p (b g)")
        sq = work.tile([128, NB * 16], F32, name=f"sq_{side}{slot}", tag="pre32")
        nrm = work.tile([128, NB * 16], F32, name=f"nrm_{side}{slot}", tag="pre32b")
        nc.vector.tensor_mul(sq, x, x)
        nc.vector.tensor_mul(nrm, y, y)
        nc.vector.tensor_tensor(nrm, sq, nrm, op=mybir.AluOpType.add)
        if side == "u":
            nc.vector.tensor_copy(pack[:, :, 0, :], nrm)
            nc.vector.tensor_sub(pack[:, :, 1, :], nrm, pack[:, :, 0, :])
            nc.vector.tensor_copy(pack[:, :, 4, :], x)
            nc.vector.tensor_sub(pack[:, :, 2, :], x, pack[:, :, 4, :])
            nc.vector.tensor_copy(pack[:, :, 5, :], y)
            nc.vector.tensor_sub(pack[:, :, 3, :], y, pack[:, :, 5, :])
        else:
            nc.vector.tensor_copy(pack[:, :, 2, :], nrm)
            nc.vector.tensor_sub(pack[:, :, 3, :], nrm, pack[:, :, 2, :])
            xhi = work.tile([128, NB * 16], F16, name=f"xhi{slot}", tag="pre16")
            xlo = work.tile([128, NB * 16], F16, name=f"xlo{slot}", tag="pre16b")
            nc.vector.tensor_copy(xhi, x)
            nc.vector.tensor_sub(xlo, x, xhi)
            nc.vector.tensor_scalar_mul(pack[:, :, 0, :], xhi, -2.0)
            nc.vector.tensor_scalar_mul(pack[:, :, 4, :], xlo, -2.0)
            yhi = work.tile([128, NB * 16], F16, name=f"yhi{slot}", tag="pre16")
            ylo = work.tile([128, NB * 16], F16, name=f"ylo{slot}", tag="pre16b")
            nc.vector.tensor_copy(yhi, y)
            nc.vector.tensor_sub(ylo, y, yhi)
            nc.vector.tensor_scalar_mul(pack[:, :, 1, :], yhi, -2.0)
            nc.vector.tensor_scalar_mul(pack[:, :, 5, :], ylo, -2.0)

    # ---- per-batch transpose + assembly of W_b, M_b [10, 2048] fp16 ----
    # K-term pairing (row k of W times row k of M):
    #   k0: nu_hi*1      k1: nu_lo*1     k2: uxlo*-2vxhi  k3: uylo*-2vyhi
    #   k4: uxhi*-2vxhi  k5: uyhi*-2vyhi k6: 1*nv_hi      k7: 1*nv_lo
    #   k8: uxhi*-2vxlo  k9: uyhi*-2vylo
    # W rows = [u0..u5, 1, 1, u4, u5]; M rows = [1, 1, v0, v1, v0, v1, v2..v5]
    # D2 column order: c = m*128 + q  <->  i = q*16 + m (consistent bijection)
    Ws, Ms = [], []
    # Batch 0's W/M gate the whole pipeline: spread its scatters over three
    # DMA queues (sync, gpsimd, scalar-HWDGE); later batches hide behind the
    # main loop on two queues.
    qs3 = [nc.sync, nc.gpsimd, nc.scalar]
    qi = 0
    for b in range(NB):
        # (dst_row_start, n_rows, src) with src None -> ones, int -> tsb row/16
        for pack, blocks, out_list, nm in (
                (pack_u, [(0, 6, 0), (8, 2, 4), (6, 2, None)], Ws, "W"),
                (pack_v, [(2, 2, 0), (4, 2, 0), (6, 4, 2), (0, 2, None)],
                 Ms, "M")):
            tp = pp.tile([96, 128], F16, name=f"tp_{nm}{b}{slot}", tag="ps", bufs=2)
            nc.tensor.transpose(tp, pack[:, b, :, :].rearrange("p a g -> p (a g)"), ident)
            tsb = work.tile([96, 128], F16, name=f"tsb_{nm}{b}{slot}", tag="tsb")
            nc.scalar.copy(tsb, tp)
            buf = work.tile([10, NPT], F16, name=f"{nm}{b}{slot}",
                            tag=f"wm{nm}{b}{slot}", bufs=1)
            for r0, nr, v in blocks:
                if b == 0:
                    eng = qs3[qi % 3]
                else:
                    eng = nc.sync if (qi % 2 == 0) else nc.gpsimd
                qi += 1
                if v is None:
                    eng.dma_start(out=buf[r0:r0 + nr, :], in_=ones_sb[:, :])
                else:
                    eng.dma_start(
                        out=buf[r0:r0 + nr, :].rearrange(
                            "r (m q) -> r m q", m=16),
                        in_=tsb[v * 16:(v + nr) * 16, :],
                    )
            out_list.append(buf)
    return Ws, Ms


def _emit_main(nc, res_d, ident, pools, Ws, Ms, slot=""):
    work, pp = pools
    # ---- main loop ----
    # The rowmin tree + colmin finalize of batch b are deferred into batch
    # b+1's pipeline (emitted at its t==2 slot) so the batch boundary never
    # stalls DVE; sqrt results accumulate per batch into sqr/sqc.
    sq2 = work.tile([128, 2, NB * NT], F32, name=f"sq2{slot}",
                    tag=f"sq2{slot}", bufs=1)
    state = {}

    pend = {}

    def emit_finalize1(b):
        colacc, s2all = state[b]
        # col-min finalize: PE transposes first so PE starts right away;
        # the PSUM reduce is deferred to emit_finalize2 so the in-order DVE
        # queue doesn't stall waiting for the transposes.
        # (DVE may read only ONE non-scalar input from PSUM, so pairwise
        # folds of pst halves are illegal; a single 1x reduce is the best.)
        pst = pp.tile([128, NT, 128], F16, name=f"pst{b}{slot}", tag="ps", bufs=2)
        for k in range(NT):
            nc.tensor.transpose(
                pst[:, k, :],
                colacc[:, 128 * k:128 * (k + 1)],
                ident,
            )
        rowm = work.tile([128, NT], F32, name=f"rowm{b}{slot}", tag=f"rm{slot}", bufs=2)
        w = NPT // 4
        while w > 16:
            nc.vector.tensor_tensor(
                s2all[:, :, :w // 2], s2all[:, :, :w // 2],
                s2all[:, :, w // 2:w], op=MIN)
            w //= 2
        nc.vector.tensor_reduce(
            out=rowm, in_=s2all[:, :, :w], axis=mybir.AxisListType.X, op=MIN)
        nc.scalar.activation(sq2[:, 0, b * NT:(b + 1) * NT], rowm,
                             mybir.ActivationFunctionType.Sqrt)
        colm = work.tile([128, NT], F32, name=f"colm{b}{slot}", tag=f"cm{slot}", bufs=2)
        nc.vector.tensor_reduce(
            out=colm, in_=pst, axis=mybir.AxisListType.X, op=MIN)
        nc.scalar.activation(sq2[:, 1, b * NT:(b + 1) * NT], colm,
                             mybir.ActivationFunctionType.Sqrt)

    for b in range(NB):
        W, M = Ws[b], Ms[b]
        colacc = work.tile([128, NPT], F16, name=f"colacc{b}{slot}",
                           tag=f"colacc{slot}", bufs=2)
        s2all = work.tile([128, NT, NPT // 4], F16, name=f"s2all{b}{slot}",
                          tag=f"s2all{slot}", bufs=2)
        state[b] = (colacc, s2all)
        cc_prev = None
        for t in range(NT):
            if b > 0 and t == 2:
                emit_finalize1(b - 1)
            if t % 2 == 0:
                cc = work.tile([128, 2, NPT], F16, name=f"cc{b}_{t}{slot}", tag="cc",
                               bufs=2)
            c = cc[:, t % 2, :]
            ps = pp.tile([128, NPT], F32, name=f"ps{b}_{t}", tag="ps", bufs=2)
            for n in range(4):
                nc.tensor.matmul(
                    ps[:, 512 * n:512 * (n + 1)],
                    W[:, 128 * t:128 * (t + 1)],
                    M[:, 512 * n:512 * (n + 1)],
                    start=True, stop=True,
                )
            nc.scalar.activation(c, ps, mybir.ActivationFunctionType.Relu)
            # ---- col-min accumulator ----
            if t == 1:
                nc.vector.tensor_tensor(colacc, cc[:, 0, :], cc[:, 1, :],
                                        op=MIN)
            elif t > 1:
                nc.vector.tensor_tensor(colacc, c, colacc, op=MIN)
            # ---- row-min: paired first-level fold + per-pair second fold ----
            if t % 2 == 1:
                s1p2 = work.tile([128, 2, NPT // 2], F16, name=f"s1p{b}_{t}{slot}",
                                 tag="s1p", bufs=2)
                nc.vector.tensor_tensor(
                    s1p2, cc[:, :, :NPT // 2], cc[:, :, NPT // 2:], op=MIN)
                nc.vector.tensor_tensor(
                    s2all[:, t - 1:t + 1, :], s1p2[:, :, :NPT // 4],
                    s1p2[:, :, NPT // 4:], op=MIN)
    emit_finalize1(NB - 1)

    # ---- epilogue: ONE fused sum over both sqrt planes (the host only
    # ever sums the partials, so row/col need not stay separate) ----
    res_sb = work.tile([128, 1], F32, name=f"res_sb{slot}", tag=f"res{slot}",
                       bufs=1)
    junk = work.tile([128, 2, NB * NT], F32, name=f"junk{slot}",
                     tag=f"junk{slot}", bufs=1)
    nc.scalar.activation(junk, sq2, mybir.ActivationFunctionType.Copy,
                         accum_out=res_sb[:, 0:1])
    nc.sync.dma_start(out=res_d[:, :], in_=res_sb)


def _emit_body(nc, out_d, tgt_d, res_d, ident, ones_sb, pools, slot=""):
    Ws, Ms = _emit_prep(nc, out_d, tgt_d, ident, ones_sb, pools, slot)
    _emit_main(nc, res_d, ident, pools, Ws, Ms, slot)


def build_kernel(reps: int = 1, staggered: bool = False):
    nc = bacc.Bacc("TRN2", target_bir_lowering=False, debug=False)
    out_d = nc.dram_tensor("outputs", [NB, 2 * NPT], F32, kind="ExternalInput")
    tgt_d = nc.dram_tensor("targets", [NB, 2 * NPT], F32, kind="ExternalInput")
    ident_d = nc.dram_tensor("ident", [128, 128], F16, kind="ExternalInput")
    ones_d = nc.dram_tensor("ones", [2, NPT], F16, kind="ExternalInput")
    res_d = nc.dram_tensor("res", [128, 1], F32, kind="ExternalOutput")
    with tile.TileContext(nc) as tc:
        with ExitStack() as ctx:
            sing = ctx.enter_context(tc.tile_pool(name="sing", bufs=1))
            work = ctx.enter_context(tc.tile_pool(name="work", bufs=6))
            pp = ctx.enter_context(tc.tile_pool(name="pp", bufs=4, space="PSUM"))
            ident, ones_sb = _emit_consts(nc, ident_d, ones_d, sing)
            pools = (work, pp)
            args = (nc, out_d, tgt_d, res_d, ident, ones_sb, pools)
            if reps == 1:
                _emit_body(*args)
            elif reps == 2:
                _emit_body(*args, slot="A")
                _emit_body(*args, slot="B")
            else:
                # Two ping-pong bodies per hardware-loop iteration: the
                # all-engine barrier in For_i's reset block then fires once
                # per TWO reps, and slot B's prep overlaps slot A's main
                # loop (and vice versa across iterations).
                # Rotating slot A's prep to the body tail (cross-edge
                # software pipelining) DEADLOCKS: Tile wait-sems for
                # iteration 2+'s main point at the prologue's prep, which
                # never re-increments after the semaphore reset.
                # staggered_reset measured SLOWER on HW than the plain
                # all-engine-barrier reset (232us vs 214us median per rep
                # at 2000 reps) - keep the regular reset.
                assert reps % 2 == 0, "reps must be even for the 2-body loop"
                with tc.For_i(0, reps // 2, 1, staggered_reset=staggered):
                    _emit_body(*args, slot="A")
                    _emit_body(*args, slot="B")
    nc.compile()
    return nc


_NC_CACHE = {}


def _get_nc(reps: int = 1):
    if reps not in _NC_CACHE:
        _NC_CACHE[reps] = build_kernel(reps)
    return _NC_CACHE[reps]


def kernel(outputs: np.ndarray, targets: np.ndarray) -> np.ndarray:
    outputs = np.ascontiguousarray(outputs, dtype=np.float32)
    targets = np.ascontiguousarray(targets, dtype=np.float32)
    ident = np.eye(128, dtype=np.float16)
    ones = np.ones((2, NPT), dtype=np.float16)
    nc = _get_nc(1)
    in_maps = [
        {
            "outputs": outputs[c * NB:(c + 1) * NB],
            "targets": targets[c * NB:(c + 1) * NB],
            "ident": ident,
            "ones": ones,
        }
        for c in range(N_CORES)
    ]
    res = run_bass_kernel_spmd(nc, in_maps, core_ids=list(range(N_CORES)))
    s = np.float64(0.0)
    for r in res.results:
        s += r["res"].astype(np.float64).sum()
    return np.float32(s * 0.5 / (NPT * NB * N_CORES))

